# revision 9
# baseline (speedup 1.0000x reference)
"""GLM-style dual-RoPE attention block on 8 trn2 NeuronCores.

Sharding: tensor-parallel over heads (16 heads -> 2 per core).
Per core: QKV projection for its heads (transposed layout), dual RoPE,
full S x S attention (streamed softmax over key tiles, no max subtraction
-- max |logit| ~60 so exp stays in fp32 range), unnormalized P@V with
matmul-ones column sums, late normalization, and a partial output
projection.  Partials are summed on host; qkv v-bias is folded into a
host-side constant row (sum_k p_k = 1), attn_out bias added on host.

All device matmuls run in float32r (full PE rate at N>=512) unless the
per-stage flags below are flipped back to fp32 (4x slower, exact).
"""

import numpy as np

import concourse.bass as bass
from concourse import bacc
import concourse.mybir as mybir
import concourse.tile as tile
from concourse.bass_utils import run_bass_kernel_spmd
from concourse.masks import make_identity

F32 = mybir.dt.float32
F32R = mybir.dt.float32r
BF16 = mybir.dt.bfloat16
AF = mybir.ActivationFunctionType

S, D, H, HD = 2048, 2048, 16, 128
NCORES = 8
HPC = H // NCORES          # heads per core = 2
KT = D // 128              # 16 contraction tiles
ST = S // 128              # 16 sequence 128-tiles
QC = S // 512              # 4 sequence 512-chunks

# Per-stage float32r enables (False -> plain fp32 matmul, 4 cyc/row)
F32R_QKV = True
F32R_ATT = True
F32R_OUT = True

_LAST_RESULTS = None
_BUILT = None


def _c(ap, on):
    return ap  # dtype now set at tile allocation (BIR requires f32r producers)


DT_IN = F32R if F32R_QKV else F32
DT_ATT = F32R if F32R_ATT else F32
DT_OUT = F32R if F32R_OUT else F32
assert F32R_ATT == F32R_OUT, "vT tile is shared between attention and out-proj"


def _build():
    nc = bacc.Bacc("TRN2", target_bir_lowering=False, debug=False,
                   num_devices=NCORES)
    xT_d = nc.dram_tensor("xT", [D, S], DT_IN, kind="ExternalInput").ap()
    wqk_d = nc.dram_tensor("wqk", [D, 4 * 128], DT_IN, kind="ExternalInput").ap()
    bqk_d = nc.dram_tensor("bqk", [128, 4], F32, kind="ExternalInput").ap()
    wv_d = nc.dram_tensor("wv", [D, HPC * 128], DT_IN, kind="ExternalInput").ap()
    cos_d = nc.dram_tensor("cos", [128, S], F32, kind="ExternalInput").ap()
    sin_d = nc.dram_tensor("sin", [128, S], F32, kind="ExternalInput").ap()
    wo_d = nc.dram_tensor("wo", [HPC * 128, D], DT_OUT, kind="ExternalInput").ap()
    out_d = nc.dram_tensor("out", [S, D], BF16, kind="ExternalOutput").ap()

    with tile.TileContext(nc) as tc:
        with (
            tc.tile_pool(name="res", bufs=1) as res,
            tc.tile_pool(name="xs", bufs=4) as xs,
            tc.tile_pool(name="tmp", bufs=2) as tmp,
            tc.tile_pool(name="ex", bufs=3) as exp_pool,
            tc.tile_pool(name="rp", bufs=2) as rp,
            tc.tile_pool(name="ob", bufs=2) as obp,
            tc.tile_pool(name="ps", bufs=8, space="PSUM") as ps,
        ):
            # ---- resident tensors ----
            wqk_sb = [res.tile([128, 512], DT_IN, tag=f"wqk{k}", name=f"wqk{k}")
                      for k in range(KT)]
            for k in range(KT):
                nc.sync.dma_start(wqk_sb[k][:], wqk_d[k * 128:(k + 1) * 128, :])
            wv_sb = [res.tile([128, 256], DT_IN, tag=f"wvw{k}", name=f"wvw{k}")
                     for k in range(KT)]
            for k in range(KT):
                nc.sync.dma_start(wv_sb[k][:], wv_d[k * 128:(k + 1) * 128, :])
            cos_sb = res.tile([128, S], F32, tag="cos")
            nc.sync.dma_start(cos_sb[:], cos_d[:, :])
            sin_sb = res.tile([128, S], F32, tag="sin")
            nc.sync.dma_start(sin_sb[:], sin_d[:, :])
            bqk_sb = res.tile([128, 4], F32, tag="bqk")
            nc.sync.dma_start(bqk_sb[:], bqk_d[:, :])
            wo_sb = [res.tile([128, D], DT_OUT, tag=f"wo{h}", name=f"wo{h}")
                     for h in range(HPC)]
            for h in range(HPC):
                nc.sync.dma_start(wo_sb[h][:], wo_d[h * 128:(h + 1) * 128, :])
            ones_f = res.tile([128, 1], F32, tag="ones_f")
            nc.gpsimd.memset(ones_f[:], 1.0)
            ones_sb = res.tile([128, 1], DT_ATT, tag="ones")
            nc.vector.tensor_copy(ones_sb[:], ones_f[:])
            ident_f = res.tile([128, 128], F32, tag="ident_f")
            make_identity(nc, ident_f[:])
            ident = res.tile([128, 128], DT_ATT, tag="ident")
            nc.vector.tensor_copy(ident[:], ident_f[:])

            qkT = [[res.tile([128, 512], DT_ATT, tag=f"qkT{m}_{nq}",
                             name=f"qkT{m}_{nq}") for nq in range(QC)]
                   for m in range(4)]
            vT = [[res.tile([128, 512], DT_ATT, tag=f"vT{h}_{nq}",
                            name=f"vT{h}_{nq}") for nq in range(QC)]
                  for h in range(HPC)]
            vnat = [res.tile([128, 256], DT_ATT, tag=f"vnat{st}",
                             name=f"vnat{st}") for st in range(ST)]

            # ---- phase 1: qkv^T = W^T @ x^T (streamed over s-quarters) ----
            for nq in range(QC):
                ns = slice(nq * 512, (nq + 1) * 512)
                psums = [ps.tile([128, 512], F32, tag="ps", name=f"qkvps{nq}_{i}") for i in range(6)]
                for k in range(KT):
                    xt = xs.tile([128, 512], DT_IN, tag="xt")
                    nc.sync.dma_start(xt[:], xT_d[k * 128:(k + 1) * 128, ns])
                    for m in range(4):
                        nc.tensor.matmul(
                            psums[m][:],
                            _c(wqk_sb[k][:, m * 128:(m + 1) * 128], F32R_QKV),
                            _c(xt[:], F32R_QKV),
                            start=(k == 0), stop=(k == KT - 1))
                    for h in range(HPC):
                        nc.tensor.matmul(
                            psums[4 + h][:],
                            _c(wv_sb[k][:, h * 128:(h + 1) * 128], F32R_QKV),
                            _c(xt[:], F32R_QKV),
                            start=(k == 0), stop=(k == KT - 1))
                # q/k: bias + rope -> qkT
                for m in range(4):
                    zb = tmp.tile([128, 512], F32, tag="zb")
                    nc.scalar.activation(zb[:], psums[m][:], AF.Identity,
                                         bias=bqk_sb[:, m:m + 1])
                    rot = tmp.tile([128, 512], F32, tag="rot")
                    for blk in range(2):
                        b0 = blk * 64
                        nc.scalar.mul(rot[b0:b0 + 32, :], zb[b0 + 32:b0 + 64, :], -1.0)
                        nc.scalar.copy(rot[b0 + 32:b0 + 64, :], zb[b0:b0 + 32, :])
                    t1 = tmp.tile([128, 512], F32, tag="t1")
                    nc.vector.tensor_mul(t1[:], zb[:], cos_sb[:, ns])
                    t2 = tmp.tile([128, 512], F32, tag="t2")
                    nc.vector.tensor_mul(t2[:], rot[:], sin_sb[:, ns])
                    nc.vector.tensor_add(qkT[m][nq][:], t1[:], t2[:])
                # v^T: plain eviction (bias folded on host)
                for h in range(HPC):
                    nc.scalar.copy(vT[h][nq][:], psums[4 + h][:])

            # ---- phase 1b: transpose v^T -> v natural [s, vd] tiles ----
            for h in range(HPC):
                for st in range(ST):
                    tp = ps.tile([128, 128], DT_ATT, tag="ps")
                    nc.tensor.transpose(
                        tp[:], vT[h][st // 4][:, (st % 4) * 128:(st % 4 + 1) * 128],
                        ident[:])
                    nc.any.tensor_copy(
                        vnat[st][:, h * 128:(h + 1) * 128], tp[:])

            wvn = vT  # reuse: vT tiles are dead after phase 1b

            # ---- phase 2: attention per (head, query-chunk) ----
            for h in range(HPC):
                qT_h = qkT[h]
                kT_h = qkT[2 + h]
                for qc in range(QC):
                    qs = slice(qc * 512, (qc + 1) * 512)
                    wv_ps = ps.tile([128, 512], F32, tag="ps")
                    sm_ps = ps.tile([1, 512], F32, tag="ps")
                    for st in range(ST):
                        lg = ps.tile([128, 512], F32, tag="ps")
                        nc.tensor.matmul(
                            lg[:],
                            _c(kT_h[st // 4][:, (st % 4) * 128:(st % 4 + 1) * 128], F32R_ATT),
                            _c(qT_h[qc][:], F32R_ATT),
                            start=True, stop=True)
                        ex = exp_pool.tile([128, 512], DT_ATT, tag="ex")
                        nc.scalar.activation(ex[:], lg[:], AF.Exp)
                        nc.tensor.matmul(
                            wv_ps[:],
                            _c(vnat[st][:, h * 128:(h + 1) * 128], F32R_ATT),
                            _c(ex[:], F32R_ATT),
                            start=(st == 0), stop=(st == ST - 1))
                        nc.tensor.matmul(
                            sm_ps[:],
                            _c(ones_sb[:], F32R_ATT),
                            _c(ex[:], F32R_ATT),
                            start=(st == 0), stop=(st == ST - 1))
                    rr = rp.tile([1, 512], F32, tag="rr")
                    nc.vector.reciprocal(rr[:], sm_ps[:])
                    rb = rp.tile([128, 512], F32, tag="rb", bufs=1)
                    nc.gpsimd.partition_broadcast(rb[:], rr[:])
                    nc.vector.tensor_mul(wvn[h][qc][:], wv_ps[:], rb[:])

            # ---- phase 3: partial out-projection [s, o] ----
            for qt in range(ST):
                ob = obp.tile([128, D], BF16, tag="ob")
                for oc in range(4):
                    op = ps.tile([128, 512], F32, tag="ps")
                    for h in range(HPC):
                        nc.tensor.matmul(
                            op[:],
                            _c(wvn[h][qt // 4][:, (qt % 4) * 128:(qt % 4 + 1) * 128], F32R_OUT),
                            _c(wo_sb[h][:, oc * 512:(oc + 1) * 512], F32R_OUT),
                            start=(h == 0), stop=(h == HPC - 1))
                    nc.any.tensor_copy(ob[:, oc * 512:(oc + 1) * 512], op[:])
                nc.sync.dma_start(out_d[qt * 128:(qt + 1) * 128, :], ob[:])

    nc.compile()
    return nc


def kernel(x, qkv_weight, qkv_bias, attn_out_weight, attn_out_bias,
           position_ids):
    global _BUILT, _LAST_RESULTS
    x = np.asarray(x, np.float32)
    qkv_weight = np.asarray(qkv_weight, np.float32)
    qkv_bias = np.asarray(qkv_bias, np.float32)
    attn_out_weight = np.asarray(attn_out_weight, np.float32)
    attn_out_bias = np.asarray(attn_out_bias, np.float32)
    position_ids = np.asarray(position_ids)

    half = HD // 2
    xT = np.ascontiguousarray(x[:, 0, :].T)
    inv_freq = 1.0 / (10000.0 ** (np.arange(0, half, 2, dtype=np.float32) / half))
    pos1 = position_ids[0, 0, :].astype(np.float32)
    pos2 = position_ids[0, 1, :].astype(np.float32)
    ang1 = np.concatenate([inv_freq[:, None] * pos1[None, :]] * 2, axis=0)
    ang2 = np.concatenate([inv_freq[:, None] * pos2[None, :]] * 2, axis=0)
    COS = np.ascontiguousarray(
        np.concatenate([np.cos(ang1), np.cos(ang2)], axis=0), dtype=np.float32)
    SIN = np.ascontiguousarray(
        np.concatenate([np.sin(ang1), np.sin(ang2)], axis=0), dtype=np.float32)

    in_maps = []
    for c in range(NCORES):
        c0 = c * HPC * HD                     # first q column of this core
        wq = qkv_weight[:, c0:c0 + HPC * HD]
        wk = qkv_weight[:, D + c0:D + c0 + HPC * HD]
        wv = qkv_weight[:, 2 * D + c0:2 * D + c0 + HPC * HD]
        bq = qkv_bias[c0:c0 + HPC * HD]
        bk = qkv_bias[D + c0:D + c0 + HPC * HD]
        wo = attn_out_weight[c0:c0 + HPC * HD, :]
        wqk = np.ascontiguousarray(np.concatenate([wq, wk], axis=1))
        bqk = np.ascontiguousarray(
            np.stack([bq[:128], bq[128:], bk[:128], bk[128:]], axis=1))
        in_maps.append({
            "xT": xT,
            "wqk": wqk,
            "bqk": bqk,
            "wv": np.ascontiguousarray(wv),
            "cos": COS,
            "sin": SIN,
            "wo": np.ascontiguousarray(wo),
        })

    if _BUILT is None:
        _BUILT = _build()
    res = run_bass_kernel_spmd(_BUILT, in_maps, core_ids=list(range(NCORES)))
    _LAST_RESULTS = res

    acc = np.zeros((S, D), dtype=np.float32)
    for r in res.results:
        acc += r["out"].astype(np.float32)
    bv = qkv_bias[2 * D:3 * D]
    acc += (bv @ attn_out_weight)[None, :] + attn_out_bias[None, :]
    return acc.reshape(S, 1, D).astype(np.float32)
